# revision 36
# baseline (speedup 1.0000x reference)
"""Trainium2 Bass kernel for nn_Conv2d_86191403696259 (single-pass conv).

Layout: consecutive-row bands. M partitions = 16 OC x 8 consecutive rows;
K partitions = (dw, input row rl 0..9, ic) -> 32*dw + 3*rl + ic (96 with gaps).
dh is folded into the band weight matrix (consecutive rows share input
partitions), so each output column needs ONE matmul pass and the lhsT is
stationary across the whole kernel.

v2 structure (per chunk of 32 output rows = 4 steps of 8):
  - DMA loads the RAW padded rows [32, 4*2050] straight into the slab's
    first partition group g0 (partitions 0-31); the dw=0 tap uses it in
    place (rhs free base = s*2050).
  - DVE makes only TWO merged 1D shifted copies per chunk:
      g1[w] = g0[w+1]  (dw=1 tap),  g2[w] = g0[w+2]  (dw=2 tap)
    each [32, 8198] fp16 at 4x rate, instead of 12 per-step copies.
  - No memsets: 2 host-zeroed gap partitions (30,31) propagate to the
    gap partitions of g1 (62,63) and g2 (94,95) through the copies.
  - ACT evacuates PSUM f32 -> fp16 stg per step; stores are issued from
    GPSIMD (SWDGE) so neither Sync (loads) nor Scalar (evac) carries them.
Output is fp16 (rel-err budget 2e-2 >> fp16 quantization) in a
partition-major layout (contiguous 16KB per partition per store) and
reassembled + upcast to f32 on the host, halving HBM store traffic.
"""

import ml_dtypes
import numpy as np

import concourse.bass as bass
import concourse.mybir as mybir
import concourse.tile as tile
from concourse import bacc
from concourse.bass_utils import run_bass_kernel_spmd

IC, OC, KH, KW = 3, 16, 3, 3
H = W = 2048
N_CORES = 8
RPC = H // N_CORES          # 256 output rows per core
HP = RPC + 2                # 258 padded input rows per core
WP = W + 2                  # 2050
S = 4                       # s-steps per chunk (8 rows each)
NCHUNK = RPC // (8 * S)     # 8
NWT = W // 512              # 4
KP = 96                     # contraction partitions (with gaps)
SW = S * WP                 # slab free width per chunk (8200)

F32 = mybir.dt.float32
FP16 = mybir.dt.float16
DT = FP16
NPDT = np.float16


def build_nc() -> bass.Bass:
    nc = bacc.Bacc("TRN2", target_bir_lowering=False, debug=False)
    xg = nc.dram_tensor("xg", [32 * NCHUNK, SW], DT, kind="ExternalInput")
    wd = nc.dram_tensor("wd", [KP, 128], DT, kind="ExternalInput")
    # blk[p, sb, w] with p = 16*j + oc, row = 8*sb + j
    blk = nc.dram_tensor("blk", [128, RPC // 8, W], DT, kind="ExternalOutput")

    with tile.TileContext(nc) as tc:
        with (
            tc.tile_pool(name="wpool", bufs=1) as wpool,
            tc.tile_pool(name="slabp", bufs=4) as slab_pool,
            tc.tile_pool(name="stgout", bufs=4) as stgout_pool,
            tc.tile_pool(name="psum", bufs=2, space="PSUM") as psum_pool,
        ):
            w_sb = wpool.tile([KP, 128], DT)

            def load_chunk(kc, st):
                if kc == 0:
                    # split first load per step (first step in halves) so
                    # copies/matmuls start as soon as data lands; the weight
                    # load is issued second (it's small + needed later)
                    nc.sync.dma_start(out=st[0:32, 0:1026], in_=xg[0:32, 0:1026])
                    nc.sync.dma_start(out=w_sb[:, :], in_=wd[:, :])
                    nc.sync.dma_start(
                        out=st[0:32, 1026:WP], in_=xg[0:32, 1026:WP]
                    )
                    nc.sync.dma_start(
                        out=st[0:32, WP : 2 * WP], in_=xg[0:32, WP : 2 * WP]
                    )
                    # s2+s3 as ONE load: each separate piece pays a ~0.6us
                    # issue + ~2us completion receipt that SERIALIZE; the
                    # merged tail lands ~1.5us earlier and unstalls the
                    # fill-phase DVE copy chain
                    nc.sync.dma_start(
                        out=st[0:32, 2 * WP : SW], in_=xg[0:32, 2 * WP : SW]
                    )
                else:
                    nc.sync.dma_start(
                        out=st[0:32, :], in_=xg[32 * kc : 32 * kc + 32, :]
                    )

            # loads are pre-issued 2 chunks ahead on the Sync ring so they
            # never wait behind the store of the current chunk (the ring is
            # FIFO); slab bufs=4 provides the matching WAR slack
            def replicate(kc, st):
                # dw replication: shifted copies of the raw group
                if kc == 0:
                    # fine-grained copies for a short pipeline fill
                    for b, e in ((0, 1024), (1024, 2048)):
                        nc.vector.tensor_copy(
                            out=st[32:64, b:e], in_=st[0:32, b + 1 : e + 1]
                        )
                        nc.vector.tensor_copy(
                            out=st[64:96, b:e], in_=st[0:32, b + 2 : e + 2]
                        )
                    for s in range(1, S):
                        b = s * WP
                        nc.vector.tensor_copy(
                            out=st[32:64, b : b + W], in_=st[0:32, b + 1 : b + 1 + W]
                        )
                        nc.vector.tensor_copy(
                            out=st[64:96, b : b + W], in_=st[0:32, b + 2 : b + 2 + W]
                        )
                elif kc in (1, 2):
                    # halved merged copies: the first halves land before the
                    # chunk's matmuls need them, closing a fill-phase gap
                    # that otherwise reappears on slow draws at chunk 1-2
                    h = 2 * WP
                    nc.vector.tensor_copy(
                        out=st[32:64, 0:h], in_=st[0:32, 1 : h + 1]
                    )
                    nc.vector.tensor_copy(
                        out=st[64:96, 0:h], in_=st[0:32, 2 : h + 2]
                    )
                    nc.vector.tensor_copy(
                        out=st[32:64, h : SW - 2], in_=st[0:32, h + 1 : SW - 1]
                    )
                    nc.vector.tensor_copy(
                        out=st[64:96, h : SW - 2], in_=st[0:32, h + 2 : SW]
                    )
                else:
                    nc.vector.tensor_copy(
                        out=st[32:64, 0 : SW - 2], in_=st[0:32, 1 : SW - 1]
                    )
                    nc.vector.tensor_copy(
                        out=st[64:96, 0 : SW - 2], in_=st[0:32, 2:SW]
                    )

            sts = [
                slab_pool.tile([KP, SW], DT, tag="slab", name=f"st{i}")
                for i in range(2)
            ]
            load_chunk(0, sts[0])
            load_chunk(1, sts[1])

            for kc in range(NCHUNK):
                st = sts[kc % 2]
                if kc + 2 < NCHUNK:
                    sts[kc % 2] = slab_pool.tile(
                        [KP, SW], DT, tag="slab", name=f"st{kc + 2}"
                    )
                    load_chunk(kc + 2, sts[kc % 2])
                replicate(kc, st)
                stg = stgout_pool.tile([128, S * W], DT, tag="stg")
                for s in range(S):
                    ps = psum_pool.tile([128, W], F32, tag="ps")
                    for wt in range(NWT):
                        nc.tensor.matmul(
                            out=ps[:, wt * 512 : (wt + 1) * 512],
                            lhsT=w_sb[:, :],
                            rhs=st[:, s * WP + wt * 512 : s * WP + wt * 512 + 512],
                            start=True,
                            stop=True,
                        )
                    # evac on ACT only: the 2-tile PSUM rotation is latency-
                    # critical and exactly matches ACT's back-to-back cadence;
                    # any DVE share of it measured slower (four variants: DVE
                    # is near-saturated by the copies, head-of-line blocks,
                    # and every PSUM tile's MMs+evac must fit the 2-step
                    # rotation deadline). First evac runs in halves so the
                    # ACT stream starts ~2us earlier.
                    o = s * W
                    if (kc == 0 and s == 0) or (kc == NCHUNK - 1 and s == S - 1):
                        # halved: at the start so the ACT stream begins ~2us
                        # earlier; at the end so the first final half-store
                        # can issue ~1us before the last evac finishes
                        nc.scalar.copy(out=stg[:, o : o + 1024], in_=ps[:, 0:1024])
                        nc.scalar.copy(
                            out=stg[:, o + 1024 : o + W], in_=ps[:, 1024:W]
                        )
                    else:
                        nc.scalar.copy(out=stg[:, o : o + W], in_=ps[:, :])
                if kc < NCHUNK - 1:
                    # store 32 rows (4 steps) in one 2MB DMA from GPSIMD
                    # (SWDGE): Sync carries only loads and Scalar only evac.
                    # (With DVE under ~60% busy the 2-port SBUF lockout that
                    # starves SWDGE descriptor generation stays mild.)
                    nc.gpsimd.dma_start(
                        out=blk[:, S * kc : S * kc + S, :], in_=stg[:, :]
                    )
                else:
                    # last chunk: store per half-step on the (now idle) Sync
                    # HWDGE ring; the very last store issues from Scalar so
                    # the final two issues run on two engines in parallel
                    for hs in range(2 * S):
                        eng = nc.scalar if hs == 2 * S - 1 else nc.sync
                        eng.dma_start(
                            out=blk[:, S * kc + hs // 2 : S * kc + hs // 2 + 1, hs % 2 * 1024 : hs % 2 * 1024 + 1024],
                            in_=stg[:, hs * 1024 : (hs + 1) * 1024],
                        )

    nc.compile()
    return nc


def make_wdiag(kernel: np.ndarray) -> np.ndarray:
    """kernel [OC, IC, KH, KW] -> stationary lhsT [KP, 128], gaps zeroed."""
    wdg = np.zeros((KP, 128), np.float32)
    for dw in range(KW):
        for j in range(8):
            for dh in range(KH):
                rl = j + dh
                for ic in range(IC):
                    wdg[32 * dw + 3 * rl + ic, 16 * j : 16 * j + OC] = kernel[
                        :, ic, dh, dw
                    ]
    return wdg


def prepare_in_maps(x: np.ndarray, kernel: np.ndarray) -> list:
    x_pad = np.zeros((IC, H + 2, W + 2), NPDT)
    x_pad[:, 1:-1, 1:-1] = x.astype(NPDT)
    wd = make_wdiag(kernel).astype(NPDT)
    # row index per (kc, s, rl): 32*kc + 8*s + rl
    rows = (
        32 * np.arange(NCHUNK)[:, None, None]
        + 8 * np.arange(S)[None, :, None]
        + np.arange(10)[None, None, :]
    )  # [NCHUNK, S, 10]
    in_maps = []
    for c in range(N_CORES):
        slab = x_pad[:, c * RPC : c * RPC + HP, :]          # [IC, HP, WP]
        g = slab[:, rows, :]                                # [IC, NCHUNK, S, 10, WP]
        g = g.transpose(1, 3, 0, 2, 4)                      # [NCHUNK, 10, IC, S, WP]
        xg = np.zeros((NCHUNK, 32, S, WP), NPDT)
        xg[:, :30, :, :] = g.reshape(NCHUNK, 30, S, WP)
        in_maps.append({"xg": xg.reshape(32 * NCHUNK, SW), "wd": wd})
    return in_maps


def gather_out(blk: np.ndarray) -> np.ndarray:
    """blk [128, RPC//8, W] (p = 16j+oc, row = 8sb+j) -> [OC, RPC, W]."""
    t = blk.reshape(8, 16, RPC // 8, W).transpose(1, 2, 0, 3)
    return t.reshape(OC, RPC, W)


_NC_CACHE = {}


def kernel(x: np.ndarray, kernel: np.ndarray) -> np.ndarray:
    assert x.shape == (IC, H, W) and kernel.shape == (OC, IC, KH, KW)
    x = np.ascontiguousarray(x, np.float32)
    kernel = np.ascontiguousarray(kernel, np.float32)

    if "nc" not in _NC_CACHE:
        _NC_CACHE["nc"] = build_nc()
    nc = _NC_CACHE["nc"]

    in_maps = prepare_in_maps(x, kernel)
    res = run_bass_kernel_spmd(nc, in_maps, core_ids=list(range(N_CORES)))
    outs = [gather_out(res.results[c]["blk"]) for c in range(N_CORES)]
    return np.concatenate(outs, axis=1).astype(np.float32)


# revision 38
# speedup vs baseline: 1.0072x; 1.0072x over previous
"""Trainium2 Bass kernel for nn_Conv2d_86191403696259 (single-pass conv).

Layout: consecutive-row bands. M partitions = 16 OC x 8 consecutive rows;
K partitions = (dw, input row rl 0..9, ic) -> 32*dw + 3*rl + ic (96 with gaps).
dh is folded into the band weight matrix (consecutive rows share input
partitions), so each output column needs ONE matmul pass and the lhsT is
stationary across the whole kernel.

Structure (per chunk of 32 output rows = 4 steps of 8):
  - DMA loads the RAW padded rows [32, 4*2050] straight into the slab's
    first partition group g0 (partitions 0-31); the dw=0 tap uses it in
    place (rhs free base = s*2050). Loads are pre-issued 2 chunks ahead
    on the Sync HWDGE ring (slab bufs=4 gives the WAR slack).
  - DVE makes only TWO merged 1D shifted copies per chunk:
      g1[w] = g0[w+1]  (dw=1 tap),  g2[w] = g0[w+2]  (dw=2 tap)
    each [32, 8198] fp16 at 4x rate, instead of 12 per-step copies.
    Chunks 0-2 split their copies finer to shorten the pipeline fill
    (the chunk 1-2 boundary otherwise stalls ~2us on slow draws).
  - No memsets: 2 host-zeroed gap partitions (30,31) propagate to the
    gap partitions of g1 (62,63) and g2 (94,95) through the copies.
  - ACT alone evacuates PSUM f32 -> fp16 stg per step, back-to-back
    (the 2-tile PSUM rotation exactly matches its cadence -- this is
    the kernel's pacing engine at ~1.97us/step); first/last evacs are
    halved to start the stream earlier / release the final stores
    sooner. Mid-kernel stores are issued from GPSIMD (SWDGE) so neither
    Sync (loads) nor Scalar (evac) carries them; the final chunk's
    half-step stores go on Sync + Scalar HWDGE (separate DMA-sem pools,
    parallel issue, lower completion latency).
Output is fp16 (rel-err budget 2e-2 >> fp16 quantization) in a
partition-major layout (contiguous 16KB per partition per store) and
reassembled + upcast to f32 on the host, halving HBM store traffic.
"""

import ml_dtypes
import numpy as np

import concourse.bass as bass
import concourse.mybir as mybir
import concourse.tile as tile
from concourse import bacc
from concourse.bass_utils import run_bass_kernel_spmd

IC, OC, KH, KW = 3, 16, 3, 3
H = W = 2048
N_CORES = 8
RPC = H // N_CORES          # 256 output rows per core
HP = RPC + 2                # 258 padded input rows per core
WP = W + 2                  # 2050
S = 4                       # s-steps per chunk (8 rows each)
NCHUNK = RPC // (8 * S)     # 8
NWT = W // 512              # 4
KP = 96                     # contraction partitions (with gaps)
SW = S * WP                 # slab free width per chunk (8200)

F32 = mybir.dt.float32
FP16 = mybir.dt.float16
DT = FP16
NPDT = np.float16


def build_nc() -> bass.Bass:
    nc = bacc.Bacc("TRN2", target_bir_lowering=False, debug=False)
    xg = nc.dram_tensor("xg", [32 * NCHUNK, SW], DT, kind="ExternalInput")
    wd = nc.dram_tensor("wd", [KP, 128], DT, kind="ExternalInput")
    # blk[p, sb, w] with p = 16*j + oc, row = 8*sb + j
    blk = nc.dram_tensor("blk", [128, RPC // 8, W], DT, kind="ExternalOutput")

    with tile.TileContext(nc) as tc:
        with (
            tc.tile_pool(name="wpool", bufs=1) as wpool,
            tc.tile_pool(name="slabp", bufs=4) as slab_pool,
            tc.tile_pool(name="stgout", bufs=4) as stgout_pool,
            tc.tile_pool(name="psum", bufs=2, space="PSUM") as psum_pool,
        ):
            w_sb = wpool.tile([KP, 128], DT)

            def load_chunk(kc, st):
                if kc == 0:
                    # split first load per step (first step in halves) so
                    # copies/matmuls start as soon as data lands; the weight
                    # load is issued second (it's small + needed later)
                    nc.sync.dma_start(out=st[0:32, 0:1026], in_=xg[0:32, 0:1026])
                    nc.sync.dma_start(out=w_sb[:, :], in_=wd[:, :])
                    nc.sync.dma_start(
                        out=st[0:32, 1026:WP], in_=xg[0:32, 1026:WP]
                    )
                    for s in range(1, S):
                        nc.sync.dma_start(
                            out=st[0:32, s * WP : (s + 1) * WP],
                            in_=xg[0:32, s * WP : (s + 1) * WP],
                        )
                else:
                    nc.sync.dma_start(
                        out=st[0:32, :], in_=xg[32 * kc : 32 * kc + 32, :]
                    )

            # loads are pre-issued 2 chunks ahead on the Sync ring so they
            # never wait behind the store of the current chunk (the ring is
            # FIFO); slab bufs=4 provides the matching WAR slack
            def replicate(kc, st):
                # dw replication: shifted copies of the raw group
                if kc == 0:
                    # fine-grained copies for a short pipeline fill
                    for b, e in ((0, 1024), (1024, 2048)):
                        nc.vector.tensor_copy(
                            out=st[32:64, b:e], in_=st[0:32, b + 1 : e + 1]
                        )
                        nc.vector.tensor_copy(
                            out=st[64:96, b:e], in_=st[0:32, b + 2 : e + 2]
                        )
                    for s in range(1, S):
                        b = s * WP
                        nc.vector.tensor_copy(
                            out=st[32:64, b : b + W], in_=st[0:32, b + 1 : b + 1 + W]
                        )
                        nc.vector.tensor_copy(
                            out=st[64:96, b : b + W], in_=st[0:32, b + 2 : b + 2 + W]
                        )
                elif kc in (1, 2):
                    # halved merged copies: the first halves land before the
                    # chunk's matmuls need them, closing a fill-phase gap
                    # that otherwise reappears on slow draws at chunk 1-2
                    h = 2 * WP
                    nc.vector.tensor_copy(
                        out=st[32:64, 0:h], in_=st[0:32, 1 : h + 1]
                    )
                    nc.vector.tensor_copy(
                        out=st[64:96, 0:h], in_=st[0:32, 2 : h + 2]
                    )
                    nc.vector.tensor_copy(
                        out=st[32:64, h : SW - 2], in_=st[0:32, h + 1 : SW - 1]
                    )
                    nc.vector.tensor_copy(
                        out=st[64:96, h : SW - 2], in_=st[0:32, h + 2 : SW]
                    )
                else:
                    nc.vector.tensor_copy(
                        out=st[32:64, 0 : SW - 2], in_=st[0:32, 1 : SW - 1]
                    )
                    nc.vector.tensor_copy(
                        out=st[64:96, 0 : SW - 2], in_=st[0:32, 2:SW]
                    )

            sts = [
                slab_pool.tile([KP, SW], DT, tag="slab", name=f"st{i}")
                for i in range(2)
            ]
            load_chunk(0, sts[0])
            load_chunk(1, sts[1])

            for kc in range(NCHUNK):
                st = sts[kc % 2]
                if kc + 2 < NCHUNK:
                    sts[kc % 2] = slab_pool.tile(
                        [KP, SW], DT, tag="slab", name=f"st{kc + 2}"
                    )
                    load_chunk(kc + 2, sts[kc % 2])
                replicate(kc, st)
                stg = stgout_pool.tile([128, S * W], DT, tag="stg")
                for s in range(S):
                    ps = psum_pool.tile([128, W], F32, tag="ps")
                    for wt in range(NWT):
                        nc.tensor.matmul(
                            out=ps[:, wt * 512 : (wt + 1) * 512],
                            lhsT=w_sb[:, :],
                            rhs=st[:, s * WP + wt * 512 : s * WP + wt * 512 + 512],
                            start=True,
                            stop=True,
                        )
                    # evac on ACT only: the 2-tile PSUM rotation is latency-
                    # critical and exactly matches ACT's back-to-back cadence;
                    # any DVE share of it measured slower (four variants: DVE
                    # is near-saturated by the copies, head-of-line blocks,
                    # and every PSUM tile's MMs+evac must fit the 2-step
                    # rotation deadline). First evac runs in halves so the
                    # ACT stream starts ~2us earlier.
                    o = s * W
                    if (kc == 0 and s == 0) or (kc == NCHUNK - 1 and s == S - 1):
                        # halved: at the start so the ACT stream begins ~2us
                        # earlier; at the end so the first final half-store
                        # can issue ~1us before the last evac finishes
                        nc.scalar.copy(out=stg[:, o : o + 1024], in_=ps[:, 0:1024])
                        nc.scalar.copy(
                            out=stg[:, o + 1024 : o + W], in_=ps[:, 1024:W]
                        )
                    else:
                        nc.scalar.copy(out=stg[:, o : o + W], in_=ps[:, :])
                if kc < NCHUNK - 1:
                    # store 32 rows (4 steps) in one 2MB DMA from GPSIMD
                    # (SWDGE): Sync carries only loads and Scalar only evac.
                    # (With DVE under ~60% busy the 2-port SBUF lockout that
                    # starves SWDGE descriptor generation stays mild.)
                    nc.gpsimd.dma_start(
                        out=blk[:, S * kc : S * kc + S, :], in_=stg[:, :]
                    )
                else:
                    # last chunk: store per half-step on the (now idle) Sync
                    # HWDGE ring; the very last store issues from Scalar so
                    # the final two issues run on two engines in parallel
                    for hs in range(2 * S):
                        eng = nc.scalar if hs == 2 * S - 1 else nc.sync
                        eng.dma_start(
                            out=blk[:, S * kc + hs // 2 : S * kc + hs // 2 + 1, hs % 2 * 1024 : hs % 2 * 1024 + 1024],
                            in_=stg[:, hs * 1024 : (hs + 1) * 1024],
                        )

    nc.compile()
    return nc


def make_wdiag(kernel: np.ndarray) -> np.ndarray:
    """kernel [OC, IC, KH, KW] -> stationary lhsT [KP, 128], gaps zeroed."""
    wdg = np.zeros((KP, 128), np.float32)
    for dw in range(KW):
        for j in range(8):
            for dh in range(KH):
                rl = j + dh
                for ic in range(IC):
                    wdg[32 * dw + 3 * rl + ic, 16 * j : 16 * j + OC] = kernel[
                        :, ic, dh, dw
                    ]
    return wdg


def prepare_in_maps(x: np.ndarray, kernel: np.ndarray) -> list:
    x_pad = np.zeros((IC, H + 2, W + 2), NPDT)
    x_pad[:, 1:-1, 1:-1] = x.astype(NPDT)
    wd = make_wdiag(kernel).astype(NPDT)
    # row index per (kc, s, rl): 32*kc + 8*s + rl
    rows = (
        32 * np.arange(NCHUNK)[:, None, None]
        + 8 * np.arange(S)[None, :, None]
        + np.arange(10)[None, None, :]
    )  # [NCHUNK, S, 10]
    in_maps = []
    for c in range(N_CORES):
        slab = x_pad[:, c * RPC : c * RPC + HP, :]          # [IC, HP, WP]
        g = slab[:, rows, :]                                # [IC, NCHUNK, S, 10, WP]
        g = g.transpose(1, 3, 0, 2, 4)                      # [NCHUNK, 10, IC, S, WP]
        xg = np.zeros((NCHUNK, 32, S, WP), NPDT)
        xg[:, :30, :, :] = g.reshape(NCHUNK, 30, S, WP)
        in_maps.append({"xg": xg.reshape(32 * NCHUNK, SW), "wd": wd})
    return in_maps


def gather_out(blk: np.ndarray) -> np.ndarray:
    """blk [128, RPC//8, W] (p = 16j+oc, row = 8sb+j) -> [OC, RPC, W]."""
    t = blk.reshape(8, 16, RPC // 8, W).transpose(1, 2, 0, 3)
    return t.reshape(OC, RPC, W)


_NC_CACHE = {}


def kernel(x: np.ndarray, kernel: np.ndarray) -> np.ndarray:
    assert x.shape == (IC, H, W) and kernel.shape == (OC, IC, KH, KW)
    x = np.ascontiguousarray(x, np.float32)
    kernel = np.ascontiguousarray(kernel, np.float32)

    if "nc" not in _NC_CACHE:
        _NC_CACHE["nc"] = build_nc()
    nc = _NC_CACHE["nc"]

    in_maps = prepare_in_maps(x, kernel)
    res = run_bass_kernel_spmd(nc, in_maps, core_ids=list(range(N_CORES)))
    outs = [gather_out(res.results[c]["blk"]) for c in range(N_CORES)]
    return np.concatenate(outs, axis=1).astype(np.float32)
